# revision 9
# baseline (speedup 1.0000x reference)
"""Single-head causal attention prefill with inline RoPE on 8 trn2 NeuronCores.

Full inputs:  x [8, 2048, 1024], Wq/Wk/Wv [64, 1024]  (all fp32)
Full outputs: (out, k, v) each [8, 2048, 64] fp32  (k is post-RoPE, v raw)

Sharding: data-parallel over batch B=8 -> one batch element per core; the
small projection weights and trig tables are replicated.

Per-core kernel design (all layouts "T" = [feature, seq] so the K=C
contraction sits on SBUF partitions):
  1. proj:   qk.T [128, T] = [Wq_perm; Wk_perm] @ x.T   (PE, PSUM accum over C)
             v.T  [64, T]  = Wv @ x.T
     Wq rows are pre-scaled by hs^-0.5 and q/k head dims are permuted to
     de-interleaved (evens, odds) order so RoPE acts on contiguous row blocks.
  2. rope:   qk_swap = Perm @ qk (PE permute-matmul swaps 32-row halves), then
             roped = qk*T1 + qk_swap*T2 with host trig tables (DVE).
  3. scores: S.T[tk, tq] tiles = k.T^T @ q.T (PE) -- transposed so softmax
             normalization needs no transpose of P; causal-empty tiles skipped.
  4. softmax: P.T = exp(S.T) on ACT (no max subtraction: |S| <~ 12 for these
             N(0,1)-scaled inputs, well within fp32 exp range); the diagonal
             128x128 block is masked multiplicatively; row sums come from an
             appended ones-column in V during the PV matmul.
  5. PV:     o.T[65, tq] += [V|1]^T @ P.T accumulated over tk in PSUM.
  6. outputs: PE-transpose k.T/v.T/o.T tiles back to natural [t, h] layout,
             un-permuting k's head dims and scaling o by 1/rowsum.

The large matmuls (projections, RoPE permute, scores, PV) run with operands
bitcast to float32r: the PE's single-pass fp32 mode, 4x the instruction rate
of exact fp32 at free-dim >= 256 with slightly reduced multiply precision.

Timing methodology (bench_device): this container has no NTFF/neuron-profile
hook, so per-execution device time is measured from wall clock by linear
scaling: build two NEFFs, one containing the kernel body once and one
containing it REPEATS times (the identical instruction stream repeated
back-to-back on device), pipeline N executions of each asynchronously through
the jit/axon queue, and take
    t_exec = (T_repeat - T_single) / (N * (REPEATS - 1)).
The subtraction cancels the axon tunnel round-trip and the per-call dispatch
overhead, both of which are properties of this client harness rather than of
the kernel; what remains is pure on-device execution time per kernel body.
"""

import numpy as np

import concourse.bass as bass
import concourse.mybir as mybir
import concourse.tile as tile
from concourse.masks import make_identity
from concourse.vector_clock import ScopedClock, VectorClock

B = 8
T = 2048
C = 1024
HS = 64
NCORES = 8
FP32 = mybir.dt.float32
F32R = mybir.dt.float32r
NT = T // 512  # 4 tq tiles of 512
NJ = T // 128  # 16 tk blocks of 128
NC_CHUNKS = C // 128  # 8 contraction chunks

class SplitDrainTileContext(tile.TileContext):
    """Walrus in this environment rejects >1 semaphore wait per instruction,
    but Tile's kernel-tail drain wants one wait per live proc. Absorb the
    global clock into the SP engine through a chain of nops first, so the
    drain itself needs no waits."""

    def _drain_and_barrier(self, tick_clock, wait_clock):
        vc = tick_clock.global_clock
        n = len(vc)
        absorbed = VectorClock([0] * n)
        for i in range(n):
            if vc[i] <= 0:
                continue
            target = absorbed.copy()
            target.require_at_least(i, vc[i])
            nop = self.nc.sync.nop()
            wait_clock.add_sem_waits(
                nop.ins,
                ScopedClock({None: target.copy()}),
                ScopedClock({None: absorbed.copy()}),
            )
            absorbed = target
        drain_inst = self.nc.sync.drain()
        wait_clock.add_sem_waits(
            drain_inst.ins,
            ScopedClock({None: tick_clock.global_clock.copy()}),
            ScopedClock({None: absorbed.copy()}),
        )
        self.nc.all_engine_barrier()
        assert self.sems is not None
        popped = self.nc._tile_sem_poison_stack.pop()
        assert popped is self._sem_poison
        self.nc.clear_and_free_semaphores(list(self.sems.allocated().values()))
        self.nc.all_engine_barrier()


def _declare_io(nc):
    return {
        "xT": nc.dram_tensor("xT", [C, T], F32R, kind="ExternalInput").ap(),
        "wqkT": nc.dram_tensor("wqkT", [C, 128], F32R, kind="ExternalInput").ap(),
        "wvT": nc.dram_tensor("wvT", [C, HS], F32R, kind="ExternalInput").ap(),
        "t1": nc.dram_tensor("t1", [128, T], FP32, kind="ExternalInput").ap(),
        "t2": nc.dram_tensor("t2", [128, T], FP32, kind="ExternalInput").ap(),
        "permT": nc.dram_tensor("permT", [128, 128], F32R, kind="ExternalInput").ap(),
        "dmask": nc.dram_tensor("dmask", [128, 128], F32R, kind="ExternalInput").ap(),
        "out": nc.dram_tensor("out", [T, HS], FP32, kind="ExternalOutput").ap(),
        "k": nc.dram_tensor("k", [T, HS], FP32, kind="ExternalOutput").ap(),
        "v": nc.dram_tensor("v", [T, HS], FP32, kind="ExternalOutput").ap(),
    }


def _emit(tc, ctx, io):
    nc = tc.nc
    xT = io["xT"]
    wqkT = io["wqkT"]
    wvT = io["wvT"]
    t1d = io["t1"]
    t2d = io["t2"]
    permTd = io["permT"]
    dmaskd = io["dmask"]
    out_d = io["out"]
    k_d = io["k"]
    v_d = io["v"]

    consts = ctx.enter_context(tc.tile_pool(name="consts", bufs=1))
    wqk_s = consts.tile([128, C], F32R, tag="wqk")  # chunk-major: [:, 128c:128c+128]
    wv_s = consts.tile([128, NC_CHUNKS * HS], F32R, tag="wv")
    t1_s = consts.tile([128, T], FP32, tag="t1")
    t2_s = consts.tile([128, T], FP32, tag="t2")
    perm_s = consts.tile([128, 128], F32R, tag="perm")
    dmask_s = consts.tile([128, 128], F32R, tag="dmask")
    ident = consts.tile([128, 128], F32R, tag="ident")
    qk_s = consts.tile([128, T], F32R, tag="qk")
    m1_s = consts.tile([128, T], FP32, tag="m1")
    m2_s = consts.tile([128, T], FP32, tag="m2")
    q_roped = consts.tile([64, T], F32R, tag="qroped")
    kv_comb = consts.tile([128, T], F32R, tag="kvcomb")  # rows 0:64 k_roped, 64:128 vT
    vones_s = consts.tile([128, NJ * (HS + 1)], F32R, tag="vones")
    kstage = consts.tile([128, NJ * HS], FP32, tag="kstage")
    ostage = consts.tile([128, NJ * HS], FP32, tag="ostage")

    # ---- phase 1: projections (+ permuted copy for rope) ----
    # DMA queue plan (3 queues): SP carries xt evens + wqk + k/out results;
    # ACT (idle until the first exp ~25us in) carries wv + xt odds; the Pool
    # SWDGE queue carries the trig tables + masks + v result. xt chunk 0 and
    # the wqk chunk DMAs are issued first so the first projection matmul can
    # start ~4us in instead of waiting out the whole preamble convoy.
    with tc.tile_pool(name="xs", bufs=4) as xs_pool, tc.tile_pool(
        name="proj_psum", bufs=8, space="PSUM"
    ) as proj_psum:
        xts = [xs_pool.tile([128, T], F32R, tag="xchunk", name=f"xt{c}") for c in range(NC_CHUNKS)]
        nc.sync.dma_start(xts[0][:, :], xT[0:128, :])
        for c in range(NC_CHUNKS):
            nc.sync.dma_start(wqk_s[:, 128 * c : 128 * (c + 1)], wqkT[128 * c : 128 * (c + 1), :])
            nc.scalar.dma_start(wv_s[:, HS * c : HS * (c + 1)], wvT[128 * c : 128 * (c + 1), :])
        for c in range(1, NC_CHUNKS):
            eng = nc.scalar if c % 2 == 1 else nc.sync
            eng.dma_start(xts[c][:, :], xT[128 * c : 128 * (c + 1), :])
        # gpsimd cannot write float32r (ALU ISA check): build the identity in
        # an fp32 scratch, then DVE-copy (f32r output rounding) into place.
        ident_f32 = consts.tile([128, 128], FP32, tag="identf")
        make_identity(nc, ident_f32[:, :])
        nc.vector.tensor_copy(ident[:, :], ident_f32[:, :])
        ones_col = consts.tile([128, 1], FP32, tag="ones")
        nc.vector.memset(ones_col[:, :], 1.0)
        for j in range(NJ):
            nc.vector.tensor_copy(
                vones_s[:, (HS + 1) * j + HS : (HS + 1) * (j + 1)], ones_col[:, :]
            )
        nc.gpsimd.dma_start(perm_s[:, :], permTd)
        nc.gpsimd.dma_start(dmask_s[:, :], dmaskd)
        nc.gpsimd.dma_start(t1_s[:, :], t1d)
        nc.gpsimd.dma_start(t2_s[:, :], t2d)

        qk_ps = [proj_psum.tile([128, 512], FP32, tag="proj", name=f"qk_ps{n}") for n in range(NT)]
        v_ps = [proj_psum.tile([64, 512], FP32, tag="proj", name=f"v_ps{n}") for n in range(NT)]
        for c in range(NC_CHUNKS):
            xt = xts[c]
            first, last = c == 0, c == NC_CHUNKS - 1
            for n in range(NT):
                nc.tensor.matmul(
                    qk_ps[n][:, :],
                    wqk_s[:, 128 * c : 128 * (c + 1)],
                    xt[:, 512 * n : 512 * (n + 1)],
                    start=first,
                    stop=last,
                )
            for n in range(NT):
                nc.tensor.matmul(
                    v_ps[n][:, :],
                    wv_s[:, HS * c : HS * (c + 1)],
                    xt[:, 512 * n : 512 * (n + 1)],
                    start=first,
                    stop=last,
                )

        # qk PSUM -> SBUF (needed as rhs of the permute matmul)
        for n in range(NT):
            nc.vector.tensor_copy(qk_s[:, 512 * n : 512 * (n + 1)], qk_ps[n][:, :])

        # qk_swap = Perm @ qk, then rope: m1 = qk*T1 (sbuf), m2 = qk_swap*T2 (psum)
        qkw_ps = [proj_psum.tile([128, 512], FP32, tag="proj", name=f"qkw_ps{n}") for n in range(NT)]
        for n in range(NT):
            sl = slice(512 * n, 512 * (n + 1))
            nc.tensor.matmul(
                qkw_ps[n][:, :], perm_s[:, :], qk_s[:, sl], start=True, stop=True
            )
            nc.vector.tensor_mul(m1_s[:, sl], qk_s[:, sl], t1_s[:, sl])
            nc.vector.tensor_mul(m2_s[:, sl], qkw_ps[n][:, :], t2_s[:, sl])
            nc.vector.tensor_add(q_roped[:, sl], m1_s[0:64, sl], m2_s[0:64, sl])
            nc.vector.tensor_add(kv_comb[0:64, sl], m1_s[64:128, sl], m2_s[64:128, sl])
            nc.vector.tensor_copy(kv_comb[64:128, sl], v_ps[n][:, :])

    # ---- phase 2: k/v natural-layout staging + [V|1] weights ----
    with tc.tile_pool(name="kv_tr", bufs=2, space="PSUM") as trp:
        for j in range(NJ):
            tr = trp.tile([128, 128], F32R, tag="tr")
            nc.tensor.transpose(tr[:, :], kv_comb[:, 128 * j : 128 * (j + 1)], ident[:, :])
            # un-de-interleave head dims: nat[2i] <- row i, nat[2i+1] <- row 32+i
            nc.vector.tensor_copy(kstage[:, HS * j : HS * (j + 1) : 2], tr[:, 0:32])
            nc.vector.tensor_copy(kstage[:, HS * j + 1 : HS * (j + 1) : 2], tr[:, 32:64])
            vsl = slice((HS + 1) * j, (HS + 1) * j + HS)
            nc.vector.tensor_copy(vones_s[:, vsl], tr[:, 64:128])
        # single batched result DMAs instead of 16 apiece
        nc.sync.dma_start(
            k_d.rearrange("(j p) h -> p j h", p=128),
            kstage[:, :].rearrange("p (j h) -> p j h", h=HS),
        )
        nc.gpsimd.dma_start(
            v_d.rearrange("(j p) h -> p j h", p=128),
            vones_s[:, :].bitcast(FP32).rearrange("p (j h) -> p j h", h=HS + 1)[:, :, 0:HS],
        )

    # ---- phase 3: attention ----
    with tc.tile_pool(name="o_psum", bufs=4, space="PSUM") as o_pool, tc.tile_pool(
        name="st_psum", bufs=3, space="PSUM"
    ) as st_pool, tc.tile_pool(name="ot_psum", bufs=1, space="PSUM") as ot_pool, tc.tile_pool(
        name="pt", bufs=6
    ) as pt_pool, tc.tile_pool(name="osb", bufs=2) as osb_pool, tc.tile_pool(
        name="rc", bufs=3
    ) as rc_pool:
        o_ps = [o_pool.tile([HS + 1, 512], FP32, tag="o", name=f"o_ps{n}") for n in range(NT)]
        for j in range(NJ):
            ksl = kv_comb[0:64, 128 * j : 128 * (j + 1)]
            i_lo = j // 4
            pts = {}
            for i in range(i_lo, NT):
                s0 = 128 * (j % 4) if i == i_lo else 0
                st = st_pool.tile([128, 512], FP32, tag="st")
                nc.tensor.matmul(
                    st[:, s0:512],
                    ksl,
                    q_roped[:, 512 * i + s0 : 512 * (i + 1)],
                    start=True,
                    stop=True,
                )
                pt = pt_pool.tile([128, 512], F32R, tag="pt")
                nc.scalar.activation(
                    pt[:, s0:512], st[:, s0:512], mybir.ActivationFunctionType.Exp
                )
                if i == i_lo:
                    nc.vector.tensor_mul(pt[:, s0 : s0 + 128], pt[:, s0 : s0 + 128], dmask_s[:, :])
                pts[i] = (pt, s0)
            for i in range(i_lo, NT):
                pt, s0 = pts[i]
                nc.tensor.matmul(
                    o_ps[i][:, s0:512],
                    vones_s[:, (HS + 1) * j : (HS + 1) * (j + 1)],
                    pt[:, s0:512],
                    start=(j == 0),
                    stop=(j == 4 * i + 3),
                )
                if j == 4 * i + 3:
                    # finalize tq tile i: transpose back + normalize by rowsum
                    osb = osb_pool.tile([HS + 1, 512], FP32, tag="osb")
                    nc.vector.tensor_copy(osb[:, :], o_ps[i][:, :])
                    for u in range(4):
                        ot = ot_pool.tile([128, HS + 1], FP32, tag="ot")
                        nc.tensor.transpose(
                            ot[:, :],
                            osb[:, 128 * u : 128 * (u + 1)],
                            ident_f32[0 : HS + 1, 0 : HS + 1],
                        )
                        rc = rc_pool.tile([128, 1], FP32, tag="rc")
                        nc.vector.reciprocal(rc[:, :], ot[:, HS : HS + 1])
                        nc.scalar.mul(
                            ostage[:, HS * (4 * i + u) : HS * (4 * i + u + 1)],
                            ot[:, 0:HS],
                            rc[:, :],
                        )
        nc.sync.dma_start(
            out_d.rearrange("(j p) h -> p j h", p=128),
            ostage[:, :].rearrange("p (j h) -> p j h", h=HS),
        )


_NC_CACHE = {}


def _split_multiwait(nc, max_w=1):
    """Walrus here rejects instructions with >1 semaphore wait. Hoist extra
    waits onto same-engine NoOps inserted immediately before the offender
    (the engine executes its stream in order, so this is semantics-preserving,
    merely stalling slightly earlier)."""
    f = nc.m.functions[0]
    blocks = list(f.blocks)
    tail = blocks[-1].instructions
    for b in blocks:
        insts = b.instructions
        fixed = []
        for inst in insts:
            si = inst.sync_info
            waits = list(si.on_wait) if si and si.on_wait else []
            if len(waits) > max_w:
                for w in waits[:-max_w]:
                    bi = nc.engines[inst.engine].nop()
                    nop = bi.ins
                    # nop() appended itself to the current (tail) block; unhook
                    for ti in range(len(tail) - 1, -1, -1):
                        if tail[ti] is nop:
                            del tail[ti]
                            break
                    nop.sync_info = mybir.SyncInfo(on_wait=[w], on_update=[])
                    fixed.append(nop)
                si.on_wait = waits[-max_w:]
            fixed.append(inst)
        if len(fixed) != len(insts):
            insts[:] = fixed


def _build_nc(repeats=1):
    if repeats in _NC_CACHE:
        return _NC_CACHE[repeats]
    from contextlib import ExitStack

    nc = bass.Bass("TRN2", target_bir_lowering=False, debug=False)
    with SplitDrainTileContext(nc) as tc, ExitStack() as outer:
        io = _declare_io(nc)
        for _ in range(repeats):
            with ExitStack() as ctx:
                _emit(tc, ctx, io)
    _split_multiwait(nc)
    _NC_CACHE[repeats] = nc
    return nc


def _host_prep(x, Wq, Wk, Wv):
    """Build the per-core input maps (host-side sharding + layout prep)."""
    x = np.asarray(x, dtype=np.float32)
    Wq = np.asarray(Wq, dtype=np.float32)
    Wk = np.asarray(Wk, dtype=np.float32)
    Wv = np.asarray(Wv, dtype=np.float32)

    scale = 1.0 / np.sqrt(HS)
    # de-interleave head dims (evens then odds) so rope acts on row blocks
    Wqp = np.concatenate([Wq[0::2], Wq[1::2]], axis=0) * scale  # [64, C]
    Wkp = np.concatenate([Wk[0::2], Wk[1::2]], axis=0)  # [64, C]
    wqkT = np.ascontiguousarray(np.concatenate([Wqp, Wkp], axis=0).T)  # [C, 128]
    wvT = np.ascontiguousarray(Wv.T)  # [C, 64]

    inv_freq = 1.0 / (10000.0 ** (np.arange(0, HS, 2, dtype=np.float32) / HS))
    t = np.arange(T, dtype=np.float32)
    freqs = np.outer(t, inv_freq)  # [T, 32]
    cos = np.cos(freqs).T.astype(np.float32)  # [32, T]
    sin = np.sin(freqs).T.astype(np.float32)
    t1 = np.concatenate([cos, cos, cos, cos], axis=0)  # [128, T]
    t2 = np.concatenate([-sin, sin, -sin, sin], axis=0)

    permT = np.zeros((128, 128), dtype=np.float32)
    for m in range(128):
        permT[m ^ 32, m] = 1.0

    p = np.arange(128)[:, None]
    c = np.arange(128)[None, :]
    dmask = (c >= p).astype(np.float32)

    shared = {
        "wqkT": wqkT,
        "wvT": wvT,
        "t1": np.ascontiguousarray(t1),
        "t2": np.ascontiguousarray(t2),
        "permT": permT,
        "dmask": dmask,
    }
    in_maps = []
    for b in range(NCORES):
        m = dict(shared)
        m["xT"] = np.ascontiguousarray(x[b].T)  # [C, T]
        in_maps.append(m)
    return in_maps


def run_device(x, Wq, Wk, Wv, trace=False, trace_cores=None):
    """Compile (cached) + run on the 8 NeuronCores. Returns ((out,k,v), raw)."""
    from concourse.bass_utils import run_bass_kernel_spmd

    nc = _build_nc()
    in_maps = _host_prep(x, Wq, Wk, Wv)
    res = run_bass_kernel_spmd(
        nc, in_maps, list(range(NCORES)), trace=trace, trace_cores=trace_cores
    )
    out = np.stack([res.results[b]["out"] for b in range(NCORES)])
    k = np.stack([res.results[b]["k"] for b in range(NCORES)])
    v = np.stack([res.results[b]["v"] for b in range(NCORES)])
    return (out, k, v), res


def kernel(x, Wq, Wk, Wv):
    (out, k, v), _ = run_device(x, Wq, Wk, Wv, trace=False)
    return out, k, v


def _make_exec(nc):
    """Build the sharded 8-core jit executor for a prebuilt Bass module.
    Returns (fn, in_names, out_names, out_avals); fn(*inputs, *outs) -> outs
    with the out buffers donated."""
    import jax
    from jax.sharding import Mesh, PartitionSpec
    from jax.experimental.shard_map import shard_map
    import concourse.bass2jax as bass2jax
    from concourse.bass2jax import _bass_exec_p, install_neuronx_cc_hook

    install_neuronx_cc_hook()

    part_name = nc.partition_id_tensor.name if nc.partition_id_tensor else None
    in_names, out_names, out_avals = [], [], []
    for alloc in nc.m.functions[0].allocations:
        if not isinstance(alloc, mybir.MemoryLocationSet):
            continue
        name = alloc.memorylocations[0].name
        if alloc.kind == "ExternalInput":
            if name != part_name:
                in_names.append(name)
        elif alloc.kind == "ExternalOutput":
            out_names.append(name)
            out_avals.append(
                jax.core.ShapedArray(tuple(alloc.tensor_shape), mybir.dt.np(alloc.dtype))
            )
    n_params = len(in_names)
    all_names = in_names + out_names
    if part_name is not None:
        all_names = all_names + [part_name]

    def _one(args, outs):
        ops = list(args) + list(outs)
        if part_name is not None:
            ops.append(bass2jax.partition_id_tensor())
        return _bass_exec_p.bind(
            *ops,
            out_avals=tuple(out_avals),
            in_names=tuple(all_names),
            out_names=tuple(out_names),
            lowering_input_output_aliases=(),
            sim_require_finite=True,
            sim_require_nnan=True,
            nc=nc,
        )

    def _body(*ops):
        args, outs = ops[:n_params], list(ops[n_params:])
        return tuple(_one(args, outs))

    devices = jax.devices()[:NCORES]
    mesh = Mesh(np.asarray(devices), ("core",))
    nin = n_params + len(out_names)
    fn = jax.jit(
        shard_map(
            _body,
            mesh=mesh,
            in_specs=(PartitionSpec("core"),) * nin,
            out_specs=(PartitionSpec("core"),) * len(out_names),
            check_rep=False,
        ),
        donate_argnums=tuple(range(n_params, nin)),
        keep_unused=True,
    )
    return fn, in_names, out_names, out_avals


BENCH_REPEATS = 9
BENCH_CALLS = 32
BENCH_ROUNDS = 5


def bench_device(x, Wq, Wk, Wv, iters=None):
    """Estimate per-execution device time of the kernel (see module docstring):
    time N pipelined executions of a 1-body NEFF and of a REPEATS-body NEFF;
    the difference divided by N*(REPEATS-1) is pure on-device time per kernel
    body, with tunnel RTT and per-call dispatch overhead cancelled.
    Returns (ns_per_exec, (out, k, v))."""
    import time

    import jax

    in_maps = _host_prep(x, Wq, Wk, Wv)
    nc1 = _build_nc(1)
    ncK = _build_nc(BENCH_REPEATS)
    fn1, in_names, out_names, out_avals = _make_exec(nc1)
    fnK, _, _, _ = _make_exec(ncK)

    concat_in = [
        np.concatenate([np.asarray(in_maps[c][nm]) for c in range(NCORES)], axis=0)
        for nm in in_names
    ]
    concat_in = [jax.device_put(a) for a in concat_in]

    def zeros():
        return [
            np.zeros((NCORES * av.shape[0], *av.shape[1:]), av.dtype) for av in out_avals
        ]

    # compile + warmup both NEFFs; grab correctness outputs from the 1-body run
    outs1 = fn1(*concat_in, *zeros())
    jax.block_until_ready(outs1)
    first = [np.asarray(o) for o in outs1]
    outsK = fnK(*concat_in, *zeros())
    jax.block_until_ready(outsK)

    def timed(fn, outs):
        t0 = time.perf_counter()
        for _ in range(BENCH_CALLS):
            outs = fn(*concat_in, *outs)
        jax.block_until_ready(outs)
        return time.perf_counter() - t0, outs

    best1 = bestK = float("inf")
    for _ in range(BENCH_ROUNDS):
        dt1, outs1 = timed(fn1, outs1)
        dtK, outsK = timed(fnK, outsK)
        best1 = min(best1, dt1)
        bestK = min(bestK, dtK)

    per_exec_s = (bestK - best1) / (BENCH_CALLS * (BENCH_REPEATS - 1))
    res = [first[i].reshape(NCORES, *out_avals[i].shape) for i in range(len(out_names))]
    by = dict(zip(out_names, res))
    return per_exec_s * 1e9, (by["out"], by["k"], by["v"])


# revision 10
# speedup vs baseline: 1.1788x; 1.1788x over previous
"""Single-head causal attention prefill with inline RoPE on 8 trn2 NeuronCores.

Full inputs:  x [8, 2048, 1024], Wq/Wk/Wv [64, 1024]  (all fp32)
Full outputs: (out, k, v) each [8, 2048, 64] fp32  (k is post-RoPE, v raw)

Sharding: data-parallel over batch B=8 -> one batch element per core; the
small projection weights and trig tables are replicated.

Per-core kernel design (all layouts "T" = [feature, seq] so the K=C
contraction sits on SBUF partitions):
  1. proj:   qk.T [128, T] = [Wq_perm; Wk_perm] @ x.T   (PE, PSUM accum over C)
             v.T  [64, T]  = Wv @ x.T
     Wq rows are pre-scaled by hs^-0.5 and q/k head dims are permuted to
     de-interleaved (evens, odds) order so RoPE acts on contiguous row blocks.
  2. rope:   qk_swap = Perm @ qk (PE permute-matmul swaps 32-row halves), then
             roped = qk*T1 + qk_swap*T2 with host trig tables (DVE).
  3. scores: S.T[tk, tq] tiles = k.T^T @ q.T (PE) -- transposed so softmax
             normalization needs no transpose of P; causal-empty tiles skipped.
  4. softmax: P.T = exp(S.T) on ACT (no max subtraction: |S| <~ 12 for these
             N(0,1)-scaled inputs, well within fp32 exp range); the diagonal
             128x128 block is masked multiplicatively; row sums come from an
             appended ones-column in V during the PV matmul.
  5. PV:     o.T[65, tq] += [V|1]^T @ P.T accumulated over tk in PSUM.
  6. outputs: PE-transpose k.T/v.T/o.T tiles back to natural [t, h] layout,
             un-permuting k's head dims and scaling o by 1/rowsum.

The large matmuls (projections, RoPE permute, scores, PV) run with operands
bitcast to float32r: the PE's single-pass fp32 mode, 4x the instruction rate
of exact fp32 at free-dim >= 256 with slightly reduced multiply precision.

Timing methodology (bench_device): this container has no NTFF/neuron-profile
hook, so per-execution device time is measured from wall clock by linear
scaling: build two NEFFs, one containing the kernel body once and one
containing it REPEATS times (the identical instruction stream repeated
back-to-back on device), pipeline N executions of each asynchronously through
the jit/axon queue, and take
    t_exec = (T_repeat - T_single) / (N * (REPEATS - 1)).
The subtraction cancels the axon tunnel round-trip and the per-call dispatch
overhead, both of which are properties of this client harness rather than of
the kernel; what remains is pure on-device execution time per kernel body.
"""

import numpy as np

import concourse.bass as bass
import concourse.mybir as mybir
import concourse.tile as tile
from concourse.masks import make_identity
from concourse.vector_clock import ScopedClock, VectorClock

B = 8
T = 2048
C = 1024
HS = 64
NCORES = 8
FP32 = mybir.dt.float32
F32R = mybir.dt.float32r
NT = T // 512  # 4 tq tiles of 512
NJ = T // 128  # 16 tk blocks of 128
NC_CHUNKS = C // 128  # 8 contraction chunks

class SplitDrainTileContext(tile.TileContext):
    """Walrus in this environment rejects >1 semaphore wait per instruction,
    but Tile's kernel-tail drain wants one wait per live proc. Absorb the
    global clock into the SP engine through a chain of nops first, so the
    drain itself needs no waits."""

    def _drain_and_barrier(self, tick_clock, wait_clock):
        vc = tick_clock.global_clock
        n = len(vc)
        absorbed = VectorClock([0] * n)
        for i in range(n):
            if vc[i] <= 0:
                continue
            target = absorbed.copy()
            target.require_at_least(i, vc[i])
            nop = self.nc.sync.nop()
            wait_clock.add_sem_waits(
                nop.ins,
                ScopedClock({None: target.copy()}),
                ScopedClock({None: absorbed.copy()}),
            )
            absorbed = target
        drain_inst = self.nc.sync.drain()
        wait_clock.add_sem_waits(
            drain_inst.ins,
            ScopedClock({None: tick_clock.global_clock.copy()}),
            ScopedClock({None: absorbed.copy()}),
        )
        self.nc.all_engine_barrier()
        assert self.sems is not None
        popped = self.nc._tile_sem_poison_stack.pop()
        assert popped is self._sem_poison
        self.nc.clear_and_free_semaphores(list(self.sems.allocated().values()))
        self.nc.all_engine_barrier()


def _declare_io(nc):
    return {
        "xT": nc.dram_tensor("xT", [C, T], F32R, kind="ExternalInput").ap(),
        "wqkT": nc.dram_tensor("wqkT", [C, 128], F32R, kind="ExternalInput").ap(),
        "wvT": nc.dram_tensor("wvT", [C, HS], F32R, kind="ExternalInput").ap(),
        "t1": nc.dram_tensor("t1", [128, T], FP32, kind="ExternalInput").ap(),
        "t2": nc.dram_tensor("t2", [128, T], FP32, kind="ExternalInput").ap(),
        "permT": nc.dram_tensor("permT", [128, 128], F32R, kind="ExternalInput").ap(),
        "dmask": nc.dram_tensor("dmask", [128, 128], F32R, kind="ExternalInput").ap(),
        "out": nc.dram_tensor("out", [T, HS], FP32, kind="ExternalOutput").ap(),
        "k": nc.dram_tensor("k", [T, HS], FP32, kind="ExternalOutput").ap(),
        "v": nc.dram_tensor("v", [T, HS], FP32, kind="ExternalOutput").ap(),
    }


def _emit(tc, ctx, io):
    nc = tc.nc
    xT = io["xT"]
    wqkT = io["wqkT"]
    wvT = io["wvT"]
    t1d = io["t1"]
    t2d = io["t2"]
    permTd = io["permT"]
    dmaskd = io["dmask"]
    out_d = io["out"]
    k_d = io["k"]
    v_d = io["v"]

    consts = ctx.enter_context(tc.tile_pool(name="consts", bufs=1))
    wqk_s = consts.tile([128, C], F32R, tag="wqk")  # chunk-major: [:, 128c:128c+128]
    wv_s = consts.tile([128, NC_CHUNKS * HS], F32R, tag="wv")
    t1_s = consts.tile([128, T], FP32, tag="t1")
    t2_s = consts.tile([128, T], FP32, tag="t2")
    perm_s = consts.tile([128, 128], F32R, tag="perm")
    dmask_s = consts.tile([128, 128], F32R, tag="dmask")
    ident = consts.tile([128, 128], F32R, tag="ident")
    qk_s = consts.tile([128, T], F32R, tag="qk")
    m1_s = consts.tile([128, T], FP32, tag="m1")
    m2_s = consts.tile([128, T], FP32, tag="m2")
    q_roped = consts.tile([64, T], F32R, tag="qroped")
    kv_comb = consts.tile([128, T], F32R, tag="kvcomb")  # rows 0:64 k_roped, 64:128 vT
    vones_s = consts.tile([128, NJ * (HS + 1)], F32R, tag="vones")
    kstage = consts.tile([128, NJ * HS], FP32, tag="kstage")
    ostage = consts.tile([128, NJ * HS], FP32, tag="ostage")

    # ---- phase 1: projections (+ permuted copy for rope) ----
    # DMA queue plan (3 queues): SP carries xt evens + wqk + k/out results;
    # ACT (idle until the first exp ~25us in) carries wv + xt odds; the Pool
    # SWDGE queue carries the trig tables + masks + v result. xt chunk 0 and
    # the wqk chunk DMAs are issued first so the first projection matmul can
    # start ~4us in instead of waiting out the whole preamble convoy.
    with tc.tile_pool(name="xs", bufs=4) as xs_pool, tc.tile_pool(
        name="proj_psum", bufs=8, space="PSUM"
    ) as proj_psum:
        xts = [xs_pool.tile([128, T], F32R, tag="xchunk", name=f"xt{c}") for c in range(NC_CHUNKS)]
        nc.sync.dma_start(xts[0][:, :], xT[0:128, :])
        for c in range(NC_CHUNKS):
            nc.sync.dma_start(wqk_s[:, 128 * c : 128 * (c + 1)], wqkT[128 * c : 128 * (c + 1), :])
            nc.scalar.dma_start(wv_s[:, HS * c : HS * (c + 1)], wvT[128 * c : 128 * (c + 1), :])
        for c in range(1, NC_CHUNKS):
            eng = nc.scalar if c % 2 == 1 else nc.sync
            eng.dma_start(xts[c][:, :], xT[128 * c : 128 * (c + 1), :])
        # gpsimd cannot write float32r (ALU ISA check): build the identity in
        # an fp32 scratch, then DVE-copy (f32r output rounding) into place.
        ident_f32 = consts.tile([128, 128], FP32, tag="identf")
        make_identity(nc, ident_f32[:, :])
        nc.vector.tensor_copy(ident[:, :], ident_f32[:, :])
        ones_col = consts.tile([128, 1], FP32, tag="ones")
        nc.vector.memset(ones_col[:, :], 1.0)
        for j in range(NJ):
            nc.vector.tensor_copy(
                vones_s[:, (HS + 1) * j + HS : (HS + 1) * (j + 1)], ones_col[:, :]
            )
        nc.sync.dma_start(perm_s[:, :], permTd)
        nc.sync.dma_start(dmask_s[:, :], dmaskd)
        nc.scalar.dma_start(t1_s[:, :], t1d)
        nc.scalar.dma_start(t2_s[:, :], t2d)

        qk_ps = [proj_psum.tile([128, 512], FP32, tag="proj", name=f"qk_ps{n}") for n in range(NT)]
        v_ps = [proj_psum.tile([64, 512], FP32, tag="proj", name=f"v_ps{n}") for n in range(NT)]
        for c in range(NC_CHUNKS):
            xt = xts[c]
            first, last = c == 0, c == NC_CHUNKS - 1
            for n in range(NT):
                nc.tensor.matmul(
                    qk_ps[n][:, :],
                    wqk_s[:, 128 * c : 128 * (c + 1)],
                    xt[:, 512 * n : 512 * (n + 1)],
                    start=first,
                    stop=last,
                )
            for n in range(NT):
                nc.tensor.matmul(
                    v_ps[n][:, :],
                    wv_s[:, HS * c : HS * (c + 1)],
                    xt[:, 512 * n : 512 * (n + 1)],
                    start=first,
                    stop=last,
                )

        # qk PSUM -> SBUF (needed as rhs of the permute matmul)
        for n in range(NT):
            nc.vector.tensor_copy(qk_s[:, 512 * n : 512 * (n + 1)], qk_ps[n][:, :])

        # qk_swap = Perm @ qk, then rope: m1 = qk*T1 (sbuf), m2 = qk_swap*T2 (psum)
        qkw_ps = [proj_psum.tile([128, 512], FP32, tag="proj", name=f"qkw_ps{n}") for n in range(NT)]
        for n in range(NT):
            sl = slice(512 * n, 512 * (n + 1))
            nc.tensor.matmul(
                qkw_ps[n][:, :], perm_s[:, :], qk_s[:, sl], start=True, stop=True
            )
            nc.vector.tensor_mul(m1_s[:, sl], qk_s[:, sl], t1_s[:, sl])
            nc.vector.tensor_mul(m2_s[:, sl], qkw_ps[n][:, :], t2_s[:, sl])
            nc.vector.tensor_add(q_roped[:, sl], m1_s[0:64, sl], m2_s[0:64, sl])
            nc.vector.tensor_add(kv_comb[0:64, sl], m1_s[64:128, sl], m2_s[64:128, sl])
            nc.vector.tensor_copy(kv_comb[64:128, sl], v_ps[n][:, :])

    # ---- phase 2: k/v natural-layout staging + [V|1] weights ----
    with tc.tile_pool(name="kv_tr", bufs=2, space="PSUM") as trp:
        for j in range(NJ):
            tr = trp.tile([128, 128], F32R, tag="tr")
            nc.tensor.transpose(tr[:, :], kv_comb[:, 128 * j : 128 * (j + 1)], ident[:, :])
            # un-de-interleave head dims: nat[2i] <- row i, nat[2i+1] <- row 32+i
            nc.vector.tensor_copy(kstage[:, HS * j : HS * (j + 1) : 2], tr[:, 0:32])
            nc.vector.tensor_copy(kstage[:, HS * j + 1 : HS * (j + 1) : 2], tr[:, 32:64])
            vsl = slice((HS + 1) * j, (HS + 1) * j + HS)
            nc.vector.tensor_copy(vones_s[:, vsl], tr[:, 64:128])
        # single batched result DMAs instead of 16 apiece
        nc.sync.dma_start(
            k_d.rearrange("(j p) h -> p j h", p=128),
            kstage[:, :].rearrange("p (j h) -> p j h", h=HS),
        )
        nc.sync.dma_start(
            v_d.rearrange("(j p) h -> p j h", p=128),
            vones_s[:, :].bitcast(FP32).rearrange("p (j h) -> p j h", h=HS + 1)[:, :, 0:HS],
        )

    # ---- phase 3: attention ----
    with tc.tile_pool(name="o_psum", bufs=4, space="PSUM") as o_pool, tc.tile_pool(
        name="st_psum", bufs=3, space="PSUM"
    ) as st_pool, tc.tile_pool(name="ot_psum", bufs=1, space="PSUM") as ot_pool, tc.tile_pool(
        name="pt", bufs=6
    ) as pt_pool, tc.tile_pool(name="osb", bufs=2) as osb_pool, tc.tile_pool(
        name="rc", bufs=3
    ) as rc_pool:
        o_ps = [o_pool.tile([HS + 1, 512], FP32, tag="o", name=f"o_ps{n}") for n in range(NT)]
        for j in range(NJ):
            ksl = kv_comb[0:64, 128 * j : 128 * (j + 1)]
            i_lo = j // 4
            pts = {}
            for i in range(i_lo, NT):
                s0 = 128 * (j % 4) if i == i_lo else 0
                st = st_pool.tile([128, 512], FP32, tag="st")
                nc.tensor.matmul(
                    st[:, s0:512],
                    ksl,
                    q_roped[:, 512 * i + s0 : 512 * (i + 1)],
                    start=True,
                    stop=True,
                )
                pt = pt_pool.tile([128, 512], F32R, tag="pt")
                nc.scalar.activation(
                    pt[:, s0:512], st[:, s0:512], mybir.ActivationFunctionType.Exp
                )
                if i == i_lo:
                    nc.vector.tensor_mul(pt[:, s0 : s0 + 128], pt[:, s0 : s0 + 128], dmask_s[:, :])
                pts[i] = (pt, s0)
            for i in range(i_lo, NT):
                pt, s0 = pts[i]
                nc.tensor.matmul(
                    o_ps[i][:, s0:512],
                    vones_s[:, (HS + 1) * j : (HS + 1) * (j + 1)],
                    pt[:, s0:512],
                    start=(j == 0),
                    stop=(j == 4 * i + 3),
                )
                if j == 4 * i + 3:
                    # finalize tq tile i: transpose back + normalize by rowsum
                    osb = osb_pool.tile([HS + 1, 512], FP32, tag="osb")
                    nc.vector.tensor_copy(osb[:, :], o_ps[i][:, :])
                    for u in range(4):
                        ot = ot_pool.tile([128, HS + 1], FP32, tag="ot")
                        nc.tensor.transpose(
                            ot[:, :],
                            osb[:, 128 * u : 128 * (u + 1)],
                            ident_f32[0 : HS + 1, 0 : HS + 1],
                        )
                        rc = rc_pool.tile([128, 1], FP32, tag="rc")
                        nc.vector.reciprocal(rc[:, :], ot[:, HS : HS + 1])
                        nc.scalar.mul(
                            ostage[:, HS * (4 * i + u) : HS * (4 * i + u + 1)],
                            ot[:, 0:HS],
                            rc[:, :],
                        )
        nc.sync.dma_start(
            out_d.rearrange("(j p) h -> p j h", p=128),
            ostage[:, :].rearrange("p (j h) -> p j h", h=HS),
        )


_NC_CACHE = {}


def _split_multiwait(nc, max_w=1):
    """Walrus here rejects instructions with >1 semaphore wait. Hoist extra
    waits onto same-engine NoOps inserted immediately before the offender
    (the engine executes its stream in order, so this is semantics-preserving,
    merely stalling slightly earlier)."""
    f = nc.m.functions[0]
    blocks = list(f.blocks)
    tail = blocks[-1].instructions
    for b in blocks:
        insts = b.instructions
        fixed = []
        for inst in insts:
            si = inst.sync_info
            waits = list(si.on_wait) if si and si.on_wait else []
            if len(waits) > max_w:
                for w in waits[:-max_w]:
                    bi = nc.engines[inst.engine].nop()
                    nop = bi.ins
                    # nop() appended itself to the current (tail) block; unhook
                    for ti in range(len(tail) - 1, -1, -1):
                        if tail[ti] is nop:
                            del tail[ti]
                            break
                    nop.sync_info = mybir.SyncInfo(on_wait=[w], on_update=[])
                    fixed.append(nop)
                si.on_wait = waits[-max_w:]
            fixed.append(inst)
        if len(fixed) != len(insts):
            insts[:] = fixed


def _build_nc(repeats=1):
    if repeats in _NC_CACHE:
        return _NC_CACHE[repeats]
    from contextlib import ExitStack

    nc = bass.Bass("TRN2", target_bir_lowering=False, debug=False)
    with SplitDrainTileContext(nc) as tc, ExitStack() as outer:
        io = _declare_io(nc)
        for _ in range(repeats):
            with ExitStack() as ctx:
                _emit(tc, ctx, io)
    _split_multiwait(nc)
    _NC_CACHE[repeats] = nc
    return nc


def _host_prep(x, Wq, Wk, Wv):
    """Build the per-core input maps (host-side sharding + layout prep)."""
    x = np.asarray(x, dtype=np.float32)
    Wq = np.asarray(Wq, dtype=np.float32)
    Wk = np.asarray(Wk, dtype=np.float32)
    Wv = np.asarray(Wv, dtype=np.float32)

    scale = 1.0 / np.sqrt(HS)
    # de-interleave head dims (evens then odds) so rope acts on row blocks
    Wqp = np.concatenate([Wq[0::2], Wq[1::2]], axis=0) * scale  # [64, C]
    Wkp = np.concatenate([Wk[0::2], Wk[1::2]], axis=0)  # [64, C]
    wqkT = np.ascontiguousarray(np.concatenate([Wqp, Wkp], axis=0).T)  # [C, 128]
    wvT = np.ascontiguousarray(Wv.T)  # [C, 64]

    inv_freq = 1.0 / (10000.0 ** (np.arange(0, HS, 2, dtype=np.float32) / HS))
    t = np.arange(T, dtype=np.float32)
    freqs = np.outer(t, inv_freq)  # [T, 32]
    cos = np.cos(freqs).T.astype(np.float32)  # [32, T]
    sin = np.sin(freqs).T.astype(np.float32)
    t1 = np.concatenate([cos, cos, cos, cos], axis=0)  # [128, T]
    t2 = np.concatenate([-sin, sin, -sin, sin], axis=0)

    permT = np.zeros((128, 128), dtype=np.float32)
    for m in range(128):
        permT[m ^ 32, m] = 1.0

    p = np.arange(128)[:, None]
    c = np.arange(128)[None, :]
    dmask = (c >= p).astype(np.float32)

    shared = {
        "wqkT": wqkT,
        "wvT": wvT,
        "t1": np.ascontiguousarray(t1),
        "t2": np.ascontiguousarray(t2),
        "permT": permT,
        "dmask": dmask,
    }
    in_maps = []
    for b in range(NCORES):
        m = dict(shared)
        m["xT"] = np.ascontiguousarray(x[b].T)  # [C, T]
        in_maps.append(m)
    return in_maps


def run_device(x, Wq, Wk, Wv, trace=False, trace_cores=None):
    """Compile (cached) + run on the 8 NeuronCores. Returns ((out,k,v), raw)."""
    from concourse.bass_utils import run_bass_kernel_spmd

    nc = _build_nc()
    in_maps = _host_prep(x, Wq, Wk, Wv)
    res = run_bass_kernel_spmd(
        nc, in_maps, list(range(NCORES)), trace=trace, trace_cores=trace_cores
    )
    out = np.stack([res.results[b]["out"] for b in range(NCORES)])
    k = np.stack([res.results[b]["k"] for b in range(NCORES)])
    v = np.stack([res.results[b]["v"] for b in range(NCORES)])
    return (out, k, v), res


def kernel(x, Wq, Wk, Wv):
    (out, k, v), _ = run_device(x, Wq, Wk, Wv, trace=False)
    return out, k, v


def _make_exec(nc):
    """Build the sharded 8-core jit executor for a prebuilt Bass module.
    Returns (fn, in_names, out_names, out_avals); fn(*inputs, *outs) -> outs
    with the out buffers donated."""
    import jax
    from jax.sharding import Mesh, PartitionSpec
    from jax.experimental.shard_map import shard_map
    import concourse.bass2jax as bass2jax
    from concourse.bass2jax import _bass_exec_p, install_neuronx_cc_hook

    install_neuronx_cc_hook()

    part_name = nc.partition_id_tensor.name if nc.partition_id_tensor else None
    in_names, out_names, out_avals = [], [], []
    for alloc in nc.m.functions[0].allocations:
        if not isinstance(alloc, mybir.MemoryLocationSet):
            continue
        name = alloc.memorylocations[0].name
        if alloc.kind == "ExternalInput":
            if name != part_name:
                in_names.append(name)
        elif alloc.kind == "ExternalOutput":
            out_names.append(name)
            out_avals.append(
                jax.core.ShapedArray(tuple(alloc.tensor_shape), mybir.dt.np(alloc.dtype))
            )
    n_params = len(in_names)
    all_names = in_names + out_names
    if part_name is not None:
        all_names = all_names + [part_name]

    def _one(args, outs):
        ops = list(args) + list(outs)
        if part_name is not None:
            ops.append(bass2jax.partition_id_tensor())
        return _bass_exec_p.bind(
            *ops,
            out_avals=tuple(out_avals),
            in_names=tuple(all_names),
            out_names=tuple(out_names),
            lowering_input_output_aliases=(),
            sim_require_finite=True,
            sim_require_nnan=True,
            nc=nc,
        )

    def _body(*ops):
        args, outs = ops[:n_params], list(ops[n_params:])
        return tuple(_one(args, outs))

    devices = jax.devices()[:NCORES]
    mesh = Mesh(np.asarray(devices), ("core",))
    nin = n_params + len(out_names)
    fn = jax.jit(
        shard_map(
            _body,
            mesh=mesh,
            in_specs=(PartitionSpec("core"),) * nin,
            out_specs=(PartitionSpec("core"),) * len(out_names),
            check_rep=False,
        ),
        donate_argnums=tuple(range(n_params, nin)),
        keep_unused=True,
    )
    return fn, in_names, out_names, out_avals


BENCH_REPEATS = 9
BENCH_CALLS = 32
BENCH_ROUNDS = 5


def bench_device(x, Wq, Wk, Wv, iters=None):
    """Estimate per-execution device time of the kernel (see module docstring):
    time N pipelined executions of a 1-body NEFF and of a REPEATS-body NEFF;
    the difference divided by N*(REPEATS-1) is pure on-device time per kernel
    body, with tunnel RTT and per-call dispatch overhead cancelled.
    Returns (ns_per_exec, (out, k, v))."""
    import time

    import jax

    in_maps = _host_prep(x, Wq, Wk, Wv)
    nc1 = _build_nc(1)
    ncK = _build_nc(BENCH_REPEATS)
    fn1, in_names, out_names, out_avals = _make_exec(nc1)
    fnK, _, _, _ = _make_exec(ncK)

    concat_in = [
        np.concatenate([np.asarray(in_maps[c][nm]) for c in range(NCORES)], axis=0)
        for nm in in_names
    ]
    concat_in = [jax.device_put(a) for a in concat_in]

    def zeros():
        return [
            np.zeros((NCORES * av.shape[0], *av.shape[1:]), av.dtype) for av in out_avals
        ]

    # compile + warmup both NEFFs; grab correctness outputs from the 1-body run
    outs1 = fn1(*concat_in, *zeros())
    jax.block_until_ready(outs1)
    first = [np.asarray(o) for o in outs1]
    outsK = fnK(*concat_in, *zeros())
    jax.block_until_ready(outsK)

    def timed(fn, outs):
        t0 = time.perf_counter()
        for _ in range(BENCH_CALLS):
            outs = fn(*concat_in, *outs)
        jax.block_until_ready(outs)
        return time.perf_counter() - t0, outs

    best1 = bestK = float("inf")
    for _ in range(BENCH_ROUNDS):
        dt1, outs1 = timed(fn1, outs1)
        dtK, outsK = timed(fnK, outsK)
        best1 = min(best1, dt1)
        bestK = min(bestK, dtK)

    per_exec_s = (bestK - best1) / (BENCH_CALLS * (BENCH_REPEATS - 1))
    res = [first[i].reshape(NCORES, *out_avals[i].shape) for i in range(len(out_names))]
    by = dict(zip(out_names, res))
    return per_exec_s * 1e9, (by["out"], by["k"], by["v"])
